# revision 1
# baseline (speedup 1.0000x reference)
"""Bernstein-basis GNN kernel for Trainium2 (8 NeuronCores).

Computes out[k] = sum_m coeffs[k,m] * Ltilde^m @ x where
Ltilde = 0.5*I - 0.5*A_norm and A_norm is the symmetric-normalized
adjacency given by (edge_row, edge_col, edge_val) via segment-sum.

Strategy: destination rows are sharded over 8 cores and re-indexed into a
"virtual row" (vrow) space chosen so that every data movement except the
source-row gather is a plain affine DMA. Each Laplacian power step is a
sparse SpMM: per 128-edge chunk, gather the 128 source rows with a
[128,1]-offset indirect DMA (one row per partition — the only indirect
form this stack supports), then matmul psum[48ch, 8rows] +=
V[128,48].T @ S[128,8] where S is a one-hot-times-value matrix for an
8-row window of destinations living on the PSUM free axis. Window
results are PE-transposed back to row-major and written with affine DMAs
both into the AllGather slab (vrow order) and into a layout that makes
the final Bernstein combination a dense 88x88 matmul (kron(I_8, coeffs)).
The host un-permutes the vrow-ordered output at the end.
"""

import os
import sys
import numpy as np
from math import comb

for _p in ("/opt/trn_rl_repo", "/root/.axon_site/_ro/trn_rl_repo"):
    if os.path.isdir(_p) and _p not in sys.path:
        sys.path.insert(0, _p)

import concourse.bass as bass
import concourse.bacc as bacc
import concourse.tile as tile
import concourse.mybir as mybir
import concourse.bass_utils as bass_utils
from concourse.masks import make_identity

NCORES = 8
K = 10          # polynomial order -> K+1 basis outputs
W = 8           # rows per window/chunk
SLOTS = 128     # edge slots per chunk (matmul contraction dim)
G = 64          # chunks per psum batch (psum free = G*W = 512)
PSF = G * W     # 512
RS8 = 8         # row-sub replication in combination kron
GC = 128        # row-groups per combination batch (batch rows = RS8*GC = 1024)
CBR = RS8 * GC  # combination batch rows (1024)

LAST_EXEC_TIME_NS = None
LAST_RESULTS = None


def _bern_coeff_matrix(k):
    m = np.zeros((k + 1, k + 1), dtype=np.float64)
    for i in range(k + 1):
        for j in range(k - i + 1):
            m[i, i + j] = ((-1) ** j) * comb(k, i) * comb(k - i, j)
    return m


def _pack_windows(rows, degs):
    """Pack rows (edge counts degs) into windows of <= W rows and <= SLOTS
    edges. Greedy two-pointer over degree-sorted rows."""
    order = np.argsort(-degs, kind="stable")
    rows_sorted = rows[order]
    degs_sorted = degs[order]
    lo, hi = 0, len(rows_sorted) - 1
    windows = []
    while lo <= hi:
        cur_rows = [int(rows_sorted[lo])]
        cur_edges = int(degs_sorted[lo])
        lo += 1
        while lo <= hi and len(cur_rows) < W and cur_edges + degs_sorted[hi] <= SLOTS:
            cur_rows.append(int(rows_sorted[hi]))
            cur_edges += int(degs_sorted[hi])
            hi -= 1
        windows.append(cur_rows)
    return windows


def _preprocess(x, edge_row, edge_col, edge_val):
    """Host-side preprocessing -> per-core data arrays + geometry.

    vrow space: core-local virtual row v in [0, NV); window wdx covers
    vrows [wdx*W, (wdx+1)*W). Global gather index of a real row r owned
    by core o at vrow v is o*NV + v.
    """
    N, C = x.shape
    E = edge_row.shape[0]
    assert N % NCORES == 0
    R = N // NCORES
    KD = K + 1
    P88 = RS8 * KD

    vals = (-0.5 * edge_val).astype(np.float64)
    if E >= N and np.array_equal(edge_row[E - N:], np.arange(N, dtype=edge_row.dtype)) \
            and np.array_equal(edge_col[E - N:], edge_row[E - N:]):
        vals[E - N:] += 0.5
        er, ec = edge_row, edge_col
    else:
        er = np.concatenate([edge_row, np.arange(N, dtype=edge_row.dtype)])
        ec = np.concatenate([edge_col, np.arange(N, dtype=edge_col.dtype)])
        vals = np.concatenate([vals, np.full(N, 0.5, np.float64)])
        E = er.shape[0]
    vals = vals.astype(np.float32)

    order = np.argsort(er, kind="stable")
    ec_s = ec[order].astype(np.int64)
    vals_s = vals[order]
    deg = np.bincount(er, minlength=N).astype(np.int64)
    rowptr = np.zeros(N + 1, dtype=np.int64)
    np.cumsum(deg, out=rowptr[1:])
    assert deg.max() <= SLOTS, "row degree exceeds chunk capacity"

    percore_windows = []
    for c in range(NCORES):
        r0, r1 = c * R, (c + 1) * R
        percore_windows.append(_pack_windows(np.arange(r0, r1), deg[r0:r1]))

    NCH = max(len(wl) for wl in percore_windows)
    NB = (NCH + G - 1) // G
    NCH = NB * G
    NV = NCH * W                         # vrows per core (incl. dummies)

    NBC = (NV + CBR - 1) // CBR          # combination batches
    NV2 = NBC * CBR                      # vrows padded for combination
    CS_ROWS = NBC * P88 * GC             # C_spread flat rows
    TPB = PSF // 128                     # 4

    # global row -> (owner, vrow) map
    vrow_of = np.full(N, -1, dtype=np.int64)
    for c in range(NCORES):
        for wdx, wrows in enumerate(percore_windows[c]):
            for lr, grow in enumerate(wrows):
                vrow_of[grow] = wdx * W + lr
    owner = np.arange(N) // R
    gidx_of = (owner * NV + vrow_of).astype(np.int32)   # gather index into vfull
    assert vrow_of.min() >= 0

    data = []
    for c in range(NCORES):
        r0 = c * R
        windows = percore_windows[c]
        cols = np.zeros((NCH, SLOTS), dtype=np.int32)    # gather idx (vrow space)
        S = np.zeros((NCH, SLOTS, W), dtype=np.float32)
        vreal = np.full(NV, -1, dtype=np.int64)          # vrow -> global row
        for wdx, wrows in enumerate(windows):
            s = 0
            for lr, grow in enumerate(wrows):
                lo, hi = rowptr[grow], rowptr[grow + 1]
                cnt = int(hi - lo)
                cols[wdx, s:s + cnt] = gidx_of[ec_s[lo:hi]]
                S[wdx, s:s + cnt, lr] = vals_s[lo:hi]
                vreal[wdx * W + lr] = grow
                s += cnt
            assert s <= SLOTS

        # x in this core's vrow order (dummies zero), used as m=0 data
        xv_own = np.zeros((NV2, C), dtype=np.float32)
        mask = vreal >= 0
        xv_own[:NV][mask] = x[vreal[mask]]

        data.append(dict(
            colsT=np.ascontiguousarray(cols.T).astype(np.int32),   # [128, NCH]
            S_T=np.ascontiguousarray(S.transpose(1, 0, 2).reshape(SLOTS, NCH * W)),
            xv_own=xv_own,
            vreal=vreal))

    # full x in global vrow order (gather source for step 1)
    xv_full = np.zeros((NCORES * NV, C), dtype=np.float32)
    for c in range(NCORES):
        vr = data[c]["vreal"]
        mask = vr >= 0
        xv_full[c * NV:(c + 1) * NV][mask] = x[vr[mask]]

    coeffs = _bern_coeff_matrix(K)
    KR = np.zeros((P88, P88), dtype=np.float32)
    for rs_ in range(RS8):
        for kk in range(KD):
            for mm in range(KD):
                KR[rs_ * KD + mm, rs_ * KD + kk] = coeffs[kk, mm]

    geom = dict(N=N, C=C, R=R, NCH=NCH, NB=NB, NV=NV, NV2=NV2, NBC=NBC,
                CS_ROWS=CS_ROWS, TPB=TPB, P88=P88)
    return geom, data, KR, xv_full


def _build_program(geom):
    """Build the SPMD bass program (identical instruction stream per core)."""
    C, NCH, NB = geom["C"], geom["NCH"], geom["NB"]
    NV, NV2, NBC = geom["NV"], geom["NV2"], geom["NBC"]
    CS_ROWS, TPB, P88 = geom["CS_ROWS"], geom["TPB"], geom["P88"]
    KD = K + 1
    NVG = NCORES * NV                 # rows of the gather source

    nc = bacc.Bacc("TRN2", target_bir_lowering=False, debug=False,
                   num_devices=NCORES)
    f32, i32 = mybir.dt.float32, mybir.dt.int32

    xv_in = nc.dram_tensor("xv", [NVG, C], f32, kind="ExternalInput").ap()
    xvo_in = nc.dram_tensor("xvo", [NV2, C], f32, kind="ExternalInput").ap()
    cols_in = nc.dram_tensor("colsT", [SLOTS, NCH], i32, kind="ExternalInput").ap()
    S_in = nc.dram_tensor("S_T", [SLOTS, NCH * W], f32, kind="ExternalInput").ap()
    KR_in = nc.dram_tensor("KR", [P88, P88], f32, kind="ExternalInput").ap()
    # out[(rs*11 + k)*NBC + b][g][c] ; host unpermutes
    out_t = nc.dram_tensor("out", [RS8 * KD * NBC * GC, C], f32,
                           kind="ExternalOutput").ap()

    with tile.TileContext(nc) as tc:
        with tc.tile_pool(name="dramv", bufs=2, space="DRAM") as dramv, \
             tc.tile_pool(name="dramp", bufs=2, space="DRAM") as dramp, \
             tc.tile_pool(name="dramc", bufs=1, space="DRAM") as dramc, \
             tc.tile_pool(name="const", bufs=1) as constp, \
             tc.tile_pool(name="psum", bufs=2, space="PSUM") as psum, \
             tc.tile_pool(name="psumt", bufs=4, space="PSUM") as psumt:

            ident = constp.tile([48, 48], f32)
            make_identity(nc, ident)
            KR_t = constp.tile([P88, P88], f32)
            nc.sync.dma_start(KR_t[:], KR_in[:])

            C_spread = dramc.tile([CS_ROWS, C], f32)

            with tc.tile_pool(name="sbufA", bufs=3) as sbuf, \
                 tc.tile_pool(name="sbufG", bufs=8) as sbufg, \
                 tc.tile_pool(name="sbufR", bufs=1) as sbufr:

                colsT_sb = sbufr.tile([SLOTS, NCH], i32)
                nc.sync.dma_start(colsT_sb[:], cols_in[:])
                S_sb = sbufr.tile([SLOTS, NCH * W], f32)
                nc.sync.dma_start(S_sb[:], S_in[:])

                # ---- m=0: place xv_own into C_spread (affine) ----
                # dest flat row for (m=0, vrow): (b*P88 + rs*KD + 0)*GC + g
                # with vrow = b*CBR + g*RS8 + rs
                for b in range(NBC):
                    src = xvo_in[b * CBR:(b + 1) * CBR, :] \
                        .rearrange("(g rs) c -> rs g c", rs=RS8)
                    dst = C_spread[b * P88 * GC:(b + 1) * P88 * GC, :] \
                        .rearrange("(p g) c -> p g c", p=P88)[::KD, :, :]
                    nc.sync.dma_start(dst, src)

                # ---- K Laplacian power steps ----
                vprev = None
                for m in range(1, K + 1):
                    p_slab = dramp.tile([NV, C], f32)
                    vsrc = xv_in if m == 1 else vprev[:, :]
                    for b in range(NB):
                        ps = psum.tile([C, PSF], f32, tag="spmm")
                        for g in range(G):
                            ch = b * G + g
                            V_t = sbufg.tile([SLOTS, C], f32, tag="vg")
                            nc.gpsimd.indirect_dma_start(
                                out=V_t[:], out_offset=None,
                                in_=vsrc,
                                in_offset=bass.IndirectOffsetOnAxis(
                                    ap=colsT_sb[:, ch:ch + 1], axis=0))
                            nc.tensor.matmul(
                                out=ps[:, g * W:(g + 1) * W],
                                lhsT=V_t[:],
                                rhs=S_sb[:, ch * W:(ch + 1) * W],
                                start=(g == 0), stop=(g == G - 1))
                        ps_sb = sbuf.tile([C, PSF], f32, tag="psdrain")
                        nc.scalar.copy(ps_sb[:], ps[:])
                        rowmaj = sbuf.tile([128, TPB * C], f32, tag="rowmaj")
                        for t in range(TPB):
                            pt = psumt.tile([128, C], f32, tag="ptrans")
                            nc.tensor.transpose(
                                out=pt[:], in_=ps_sb[:, t * 128:(t + 1) * 128],
                                identity=ident[:])
                            nc.scalar.copy(rowmaj[:, t * C:(t + 1) * C], pt[:])
                        # slab write (affine): vrow = b*PSF + t*128 + p
                        nc.sync.dma_start(
                            p_slab[b * PSF:(b + 1) * PSF, :]
                            .rearrange("(t p) c -> p t c", p=128),
                            rowmaj[:].rearrange("p (t c) -> p t c", t=TPB))
                        # C_spread write (affine):
                        # flat = (b2*P88 + rs*KD + m)*GC + g2 ; vrow = b*PSF+t*128+p
                        # b2 = b//2 ; rs = p%8 ; g2 = (b%2)*64 + t*16 + p//8
                        base = ((b // 2) * P88 + m) * GC + (b % 2) * 64
                        cs_ap = C_spread[:, :]
                        for t in range(TPB):
                            dst = bass.AP(
                                cs_ap.tensor,
                                cs_ap.offset + (base + t * 16) * C,
                                [[C, 16], [KD * GC * C, 8], [1, C]])
                            nc.sync.dma_start(dst, rowmaj[:, t * C:(t + 1) * C])
                    if m < K:
                        vnew = dramv.tile([NVG, C], f32, addr_space="Shared")
                        nc.gpsimd.collective_compute(
                            "AllGather", mybir.AluOpType.bypass,
                            replica_groups=[list(range(NCORES))],
                            ins=[p_slab[:, :]],
                            outs=[vnew[:, :]])
                        vprev = vnew

            # ---- Bernstein combination ----
            with tc.tile_pool(name="sbufB", bufs=2) as sbufb:
                for b in range(NBC):
                    rhs = sbufb.tile([P88, GC * C], f32, tag="crhs")
                    nc.sync.dma_start(
                        rhs[:],
                        C_spread[b * P88 * GC:(b + 1) * P88 * GC, :]
                        .rearrange("(p g) c -> p (g c)", p=P88))
                    outb = sbufb.tile([P88, GC * C], f32, tag="cout")
                    nmm = (GC * C + 511) // 512
                    for j in range(nmm):
                        f0 = j * 512
                        f1 = min(f0 + 512, GC * C)
                        cp = psum.tile([P88, 512], f32, tag="cpsum")
                        nc.tensor.matmul(out=cp[:, :f1 - f0], lhsT=KR_t[:],
                                         rhs=rhs[:, f0:f1], start=True, stop=True)
                        nc.scalar.copy(outb[:, f0:f1], cp[:, :f1 - f0])
                    # affine output write: row (rs*KD + k)*NBC + b, free (g, c)
                    dst = bass.AP(
                        out_t.tensor,
                        out_t.offset + b * GC * C,
                        [[KD * NBC * GC * C, RS8], [NBC * GC * C, KD], [1, GC * C]])
                    nc.sync.dma_start(dst, outb[:])

    nc.compile()
    return nc


def _make_in_maps(data, KR, xv_full):
    in_maps = []
    for d in data:
        in_maps.append({
            "xv": xv_full,
            "xvo": d["xv_own"],
            "colsT": d["colsT"],
            "S_T": d["S_T"],
            "KR": KR,
        })
    return in_maps


def _ensure_ntff_hook():
    """The agent image's antenv lacks axon_hooks; provide it so
    run_bass_kernel_spmd(trace=True) can capture NTFF profiles."""
    try:
        from antenv.axon_hooks import get_axon_ntff_profile_hook  # noqa: F401
        return True
    except ImportError:
        pass
    try:
        import types
        import antenv
        from trn_agent_boot.trn_boot import _ntff_profile_via_ctypes
        mod = types.ModuleType("antenv.axon_hooks")
        _hook = [None]
        mod.set_axon_ntff_profile_hook = lambda h: _hook.__setitem__(0, h)
        mod.get_axon_ntff_profile_hook = lambda: _hook[0]
        sys.modules["antenv.axon_hooks"] = mod
        antenv.axon_hooks = mod
        mod.set_axon_ntff_profile_hook(
            _ntff_profile_via_ctypes("/opt/axon/libaxon_pjrt.so"))
        return True
    except Exception:
        return False


def kernel(x, edge_row, edge_col, edge_val):
    global LAST_EXEC_TIME_NS, LAST_RESULTS
    x = np.ascontiguousarray(np.asarray(x, dtype=np.float32))
    edge_row = np.asarray(edge_row, dtype=np.int32)
    edge_col = np.asarray(edge_col, dtype=np.int32)
    edge_val = np.asarray(edge_val, dtype=np.float32)
    N, C = x.shape
    R = N // NCORES
    KD = K + 1

    geom, data, KR, xv_full = _preprocess(x, edge_row, edge_col, edge_val)
    nc = _build_program(geom)
    in_maps = _make_in_maps(data, KR, xv_full)

    trace = bool(os.environ.get("BASS_TRACE"))
    if trace:
        trace = _ensure_ntff_hook()
    res = bass_utils.run_bass_kernel_spmd(
        nc, in_maps, core_ids=list(range(NCORES)), trace=trace)
    LAST_RESULTS = res
    LAST_EXEC_TIME_NS = res.exec_time_ns

    NBC, NV = geom["NBC"], geom["NV"]
    out = np.empty((KD, N, C), dtype=np.float32)
    for c in range(NCORES):
        raw = res.results[c]["out"].reshape(RS8, KD, NBC, GC, C)
        vr = data[c]["vreal"]
        mask = vr >= 0
        vrows = np.nonzero(mask)[0]
        bb = vrows // CBR
        gg = (vrows % CBR) // RS8
        rs = vrows % RS8
        out[:, vr[mask], :] = raw[rs, :, bb, gg, :].transpose(1, 0, 2)
    return out

